# revision 1
# baseline (speedup 1.0000x reference)
"""Cost-volume block kernel for Trainium2 (8 NeuronCores, batch-sharded).

Computes, for c1/warp of shape [B, H, W, C] (B=8, H=192, W=640, C=32):
    cost[d] = mean_c( c1[..., c] * warp_shifted_by(d-2)[..., c] )   d in 0..4
    out     = concat([c1, cost_0..cost_4], axis=-1)                 # [B,H,W,37]

Strategy:
  - one batch per NeuronCore (8 cores), SPMD program via run_bass_kernel_spmd.
  - host-side shard prep: warp is repacked to [H, 2, 324, C] half-rows, each
    carrying its 2-pixel halo (neighbor pixels, zeros at the true row edges).
    This makes every device DMA a plain 2D access pattern (partition = one
    DRAM-ordered half-row, contiguous free dim) — the shape SWDGE moves at
    ~300 GB/s — and removes all edge cases from the device program.
  - per core, partition dim = 128 consecutive half-rows (64 h rows x 2),
    free dim = w-chunk pixels x 32 channels.
  - products + channel-sum fused into ONE DVE pass with a custom DVE op:
        scanout[k] = cumsum(c1[k] * warp[k]) * (1/32)
    then per-pixel channel sums are strided differences of the prefix sums at
    32-element boundaries (one cheap tensor_sub per offset, on GpSimd).
  - the 5 shift offsets are free-dim slices of the haloed warp window.
  - the device emits only the derived cost volume [H, W, 5]; the c1
    passthrough channels of the output are assembled host-side during the
    gather/unshard step (c1 is returned bit-exact).
"""

import sys

if "/opt/trn_rl_repo" not in sys.path:
    sys.path.insert(0, "/opt/trn_rl_repo")

import numpy as np

# Problem constants (hardcoded per harness contract).
B, H, W, C = 8, 192, 640, 32
SR = 2                  # search range
NOFF = 2 * SR + 1       # 5 disparity offsets
OUTC = C + NOFF         # 37 output channels

HB = 3                  # h blocks of 64 rows = 128 half-rows
WHALF = W // 2          # 320 pixels per half-row
WHALO = WHALF + 2 * SR  # 324 pixels per haloed half-row
# (start, width) w-chunks per half-row
CHUNKS = [(0, 80), (80, 80), (160, 80), (240, 80)]
WCMAX = max(w for _, w in CHUNKS)
F = WCMAX * C                # 2560 free elements (c1 / scan tile size)
FH = (WCMAX + 2 * SR) * C    # 2688 free elements (warp window with halo)

USE_CUSTOM_OP = True
DEVICE_FULL_OUTPUT = False   # False: device writes cost[H,W,5]; host concats c1

_BUILT = None           # (nc, mulscan_op)


def _register_mulscan():
    """Register the fused multiply+prefix-scan custom DVE op at runtime."""
    import concourse.dve_ops as dvo
    from concourse.dve_spec import Spec, Src0, Src1, C2, AluOp, scan, lower, _has_src1
    from concourse.dve_uop import DveOpSpec

    name = "MULSCAN_CV"
    if name in dvo._SUB_OPCODE_FOR_NAME:
        return next(op for op in dvo.OPS if op.name == name)

    def _ref(in0, in1, s0, s1, imm2):
        return np.cumsum(
            (in0.astype(np.float32) * in1.astype(np.float32)),
            axis=-1, dtype=np.float32,
        ) * np.float32(imm2)

    spec = Spec(body=scan(AluOp.ADD, Src0 * Src1) * C2, reference=_ref)
    opcode = dvo._CUSTOM_DVE_ROW_BASE + len(dvo.OPS)
    shas = {}
    for ver in ("v3", "v4"):
        try:
            s = DveOpSpec(name=name, opcode=opcode, uops=lower(spec, ver=ver),
                          rd1_en=_has_src1(spec))
            shas[ver] = s.sha(ver)
        except Exception:
            pass
    op = dvo.DveOp(name, spec, subdim=False, uops_sha=shas)
    dvo.OPS.append(op)
    dvo._SUB_OPCODE_FOR_NAME[name] = opcode
    dvo.CUSTOM_DVE_SPECS[name] = spec
    return op


def _build():
    """Build + schedule the per-core Bass program (shapes are per-core)."""
    global _BUILT
    if _BUILT is not None:
        return _BUILT

    import concourse.bacc as bacc
    import concourse.mybir as mybir
    import concourse.tile as tile

    mulscan = _register_mulscan() if USE_CUSTOM_OP else None

    f32 = mybir.dt.float32
    nc = bacc.Bacc("TRN2", target_bir_lowering=False, debug=False)
    c1 = nc.dram_tensor("c1", [H, W, C], f32, kind="ExternalInput").ap()
    warph = nc.dram_tensor("warph", [H, 2, WHALO, C], f32,
                           kind="ExternalInput").ap()
    oc = OUTC if DEVICE_FULL_OUTPUT else NOFF
    out = nc.dram_tensor("out", [H, W, oc], f32, kind="ExternalOutput").ap()

    # Flat half-row views: [hb, 128 half-rows, row-contiguous free dim].
    c1_f = c1.rearrange("(hb h) (r w) c -> hb (h r) (w c)", hb=HB, r=2)
    wp_f = warph.rearrange("(hb h) r w c -> hb (h r) (w c)", hb=HB)
    out_f = out.rearrange("(hb h) (r w) c -> hb (h r) (w c)", hb=HB, r=2)

    with tile.TileContext(nc) as tc:
        with tc.tile_pool(name="ins", bufs=7) as ins, \
             tc.tile_pool(name="outs", bufs=2) as outs, \
             tc.tile_pool(name="work", bufs=3) as wk:
            for hb in range(HB):
                # cost for the whole h-block accumulates here
                out_t = outs.tile([128, WHALF * oc], f32, tag="out")
                out_pix = out_t[:].rearrange("p (w c) -> p w c", c=oc)
                for (w0, wcw) in CHUNKS:
                    fc = wcw * C             # c1/scan elements this chunk
                    fhc = (wcw + 2 * SR) * C  # warp window elements
                    c1_t = ins.tile([128, F], f32, tag="c1")
                    wp_t = ins.tile([128, FH], f32, tag="wp")

                    # --- loads (plain 2D APs, contiguous per partition) ------
                    nc.gpsimd.dma_start(
                        out=c1_t[:, 0:fc],
                        in_=c1_f[hb][:, w0 * C:w0 * C + fc])
                    nc.gpsimd.dma_start(
                        out=wp_t[:, 0:fhc],
                        in_=wp_f[hb][:, w0 * C:w0 * C + fhc])

                    cbase = C if DEVICE_FULL_OUTPUT else 0
                    if DEVICE_FULL_OUTPUT:
                        c1_pix = c1_t[:, 0:fc].rearrange("p (w c) -> p w c", c=C)
                        nc.scalar.copy(out=out_pix[:, w0:w0 + wcw, 0:C],
                                       in_=c1_pix[:, :, :])

                    # --- fused multiply + prefix scan + strided diff ---------
                    if USE_CUSTOM_OP:
                        scan_t = wk.tile([128, 1 + F], f32, tag="scan")
                        nc.gpsimd.memset(scan_t[:, 0:1], 0.0)
                        hi = scan_t[:, 1:1 + fc].rearrange("p (s c) -> p s c", c=C)
                        lo = scan_t[:, 0:fc].rearrange("p (s c) -> p s c", c=C)
                        for d in range(NOFF):
                            nc.vector._custom_dve(
                                mulscan,
                                out=scan_t[:, 1:1 + fc],
                                in0=c1_t[:, 0:fc],
                                in1=wp_t[:, d * C:d * C + fc],
                                imm2=1.0 / C,
                            )
                            # strided diff on GpSimd so the DVE streams scans
                            nc.gpsimd.tensor_sub(
                                out=out_pix[:, w0:w0 + wcw,
                                            cbase + d:cbase + d + 1],
                                in0=hi[:, :, C - 1:C],
                                in1=lo[:, :, 0:1],
                            )
                    else:
                        prod_t = wk.tile([128, F], f32, tag="prod")
                        for d in range(NOFF):
                            nc.vector.scalar_tensor_tensor(
                                out=prod_t[:, 0:fc],
                                in0=c1_t[:, 0:fc],
                                scalar=1.0 / C,
                                in1=wp_t[:, d * C:d * C + fc],
                                op0=mybir.AluOpType.mult,
                                op1=mybir.AluOpType.mult,
                            )
                            nc.vector.tensor_reduce(
                                out=out_pix[:, w0:w0 + wcw,
                                            cbase + d:cbase + d + 1],
                                in_=prod_t[:, 0:fc].rearrange(
                                    "p (s c) -> p s c", c=C),
                                axis=mybir.AxisListType.X,
                                op=mybir.AluOpType.add,
                            )

                    # --- store this wc's columns (2D AP, overlaps compute) ---
                    oslice = slice(w0 * oc, (w0 + wcw) * oc)
                    nc.sync.dma_start(out=out_f[hb][:, oslice],
                                      in_=out_t[:, oslice])

    nc.compile()
    _BUILT = (nc, mulscan)
    return _BUILT


def _prep_warph(warp):
    """[B, H, W, C] -> haloed half-rows [B, H, 2, 324, C] (host-side)."""
    wh = np.zeros((B, H, 2, WHALO, C), dtype=np.float32)
    wh[:, :, 0, SR:SR + WHALF] = warp[:, :, :WHALF]
    wh[:, :, 1, SR:SR + WHALF] = warp[:, :, WHALF:]
    # halos: interior neighbors; true row edges stay zero
    wh[:, :, 0, SR + WHALF:] = warp[:, :, WHALF:WHALF + SR]          # w 320,321
    wh[:, :, 1, :SR] = warp[:, :, WHALF - SR:WHALF]                  # w 318,319
    return wh


def _run(c1_full, warph_full, trace=False, **kw):
    from concourse.bass_utils import run_bass_kernel_spmd

    nc, _ = _build()
    in_maps = [{"c1": c1_full[i], "warph": warph_full[i]} for i in range(B)]
    return run_bass_kernel_spmd(nc, in_maps, list(range(B)), trace=trace, **kw)


def kernel(c1, warp, search_range):
    assert int(search_range) == SR, f"kernel hardcodes search_range={SR}"
    c1 = np.ascontiguousarray(np.asarray(c1, dtype=np.float32))
    warp = np.ascontiguousarray(np.asarray(warp, dtype=np.float32))
    assert c1.shape == (B, H, W, C) and warp.shape == (B, H, W, C)
    warph = _prep_warph(warp)
    r = _run(c1, warph, trace=False)
    if DEVICE_FULL_OUTPUT:
        return np.stack([r.results[i]["out"] for i in range(B)], axis=0)
    out = np.empty((B, H, W, OUTC), dtype=np.float32)
    out[..., :C] = c1
    for i in range(B):
        out[i, ..., C:] = r.results[i]["out"]
    return out



# revision 2
# speedup vs baseline: 1.8706x; 1.8706x over previous
"""Cost-volume block kernel for Trainium2 (8 NeuronCores, batch-sharded).

Computes, for c1/warp of shape [B, H, W, C] (B=8, H=192, W=640, C=32):
    cost[d] = mean_c( c1[..., c] * warp_shifted_by(d-2)[..., c] )   d in 0..4
    out     = concat([c1, cost_0..cost_4], axis=-1)                 # [B,H,W,37]

Strategy (v2 — multi-engine split, bf16):
  - one batch per NeuronCore (8 cores), SPMD via run_bass_kernel_spmd.
  - host-side: inputs cast to bf16 and transposed to a channels-on-partition
    layout: partition p = r*32 + c (r = row-within-4-row-subgroup, c = chan),
    free dim = (subgroup g in 0..3, pixel w).  Each "big group" G covers 16
    image rows = 4 subgroups x 4 rows; 12 groups per core.
  - DVE does the only elementwise work: 5 bf16 tensor_tensor multiplies per
    group (one per disparity offset), running in 2x_1P packed mode.  Odd
    offsets read a 1-pixel-shifted copy of the warp tile so every operand
    stays 4B-aligned (2x mode requirement).
  - TensorE reduces channels: matmul with a sparse block-ones stationary
    (value 1/32, folding the channel mean) contracts the 128-partition dim;
    each (g, d) pair has its own [128, 80] stationary whose single non-zero
    column block routes the 4 row-sums to psum partition m = g*20 + r*5 + d.
    All 20 (g, d) matmuls accumulate into one [80, W] psum tile per group.
  - ScalarE evacuates PSUM -> SBUF; all DMA on HWDGE (sync for loads,
    scalar for stores).  GpSimd is not used at all.
  - host gathers [12, 80, 640] f32 per core back to [H, W, 5] and concats
    the c1 passthrough channels (bit-exact, host-side).
"""

import sys

if "/opt/trn_rl_repo" not in sys.path:
    sys.path.insert(0, "/opt/trn_rl_repo")

import numpy as np
from ml_dtypes import bfloat16

# Problem constants (hardcoded per harness contract).
B, H, W, C = 8, 192, 640, 32
SR = 2                  # search range
NOFF = 2 * SR + 1       # 5 disparity offsets
OUTC = C + NOFF         # 37 output channels

NG = 12                 # big groups per core (16 rows each)
NSUB = 4                # subgroups (g) per big group
NR = 4                  # rows (r) per subgroup; partition p = r*32 + c
WB = W + 2 * SR + 4     # 648: padded halo width per subgroup block
FW = NSUB * W           # 2560 free elems per c1 / product tile
FWP = NSUB * WB         # 2592 free elems per warp tile
M = NSUB * NR * NOFF    # 80 psum partitions: m = g*20 + r*5 + d
NH = 2                  # psum bank halves per group (matmul N = 320)
NCHUNK = W // NH        # 320

USE_FUSED_MULT = True   # one 3D-AP multiply per (G, d) vs 4 separate 2D ones

_BUILT = None


def _build():
    """Build + schedule the per-core Bass program (shapes are per-core)."""
    global _BUILT
    if _BUILT is not None:
        return _BUILT

    import concourse.bacc as bacc
    import concourse.mybir as mybir
    import concourse.tile as tile

    f32 = mybir.dt.float32
    bf16 = mybir.dt.bfloat16
    nc = bacc.Bacc("TRN2", target_bir_lowering=False, debug=False)
    c1T = nc.dram_tensor("c1t", [NG, 128, FW], bf16, kind="ExternalInput").ap()
    wpT = nc.dram_tensor("wpt", [NG, 128, FWP], bf16, kind="ExternalInput").ap()
    sON = nc.dram_tensor("sones", [128, NSUB * NOFF * M], bf16,
                         kind="ExternalInput").ap()
    out = nc.dram_tensor("out", [NG, M, W], f32, kind="ExternalOutput").ap()

    with tile.TileContext(nc) as tc:
        with tc.tile_pool(name="const", bufs=1) as cons, \
             tc.tile_pool(name="ins", bufs=3) as ins, \
             tc.tile_pool(name="prod", bufs=2) as pr, \
             tc.tile_pool(name="psum", bufs=2, space="PSUM") as pp, \
             tc.tile_pool(name="outs", bufs=2) as outs:
            s_t = cons.tile([128, NSUB * NOFF * M], bf16)
            nc.sync.dma_start(out=s_t, in_=sON)
            for G in range(NG):
                c1_t = ins.tile([128, FW], bf16, tag="c1")
                we_t = ins.tile([128, FWP], bf16, tag="we")
                wo_t = ins.tile([128, FWP], bf16, tag="wo")
                nc.sync.dma_start(out=c1_t, in_=c1T[G])
                nc.sync.dma_start(out=we_t, in_=wpT[G])
                # odd-alignment copy: wo[j] = wp[j+1], keeps odd-d operands
                # 4B-aligned so the DVE multiply stays in 2x packed mode
                nc.sync.dma_start(out=wo_t[:, 0:FWP - 1],
                                  in_=wpT[G][:, 1:FWP])
                # each half h lives in its own psum bank (matmul can't cross)
                ps_t = pp.tile([M, NH, 512], f32)
                c1_3 = c1_t[:].rearrange("p (g w) -> p g w", g=NSUB)
                for d in range(NOFF):
                    src, off = (we_t, d) if d % 2 == 0 else (wo_t, d - 1)
                    p_t = pr.tile([128, FW], bf16, tag=f"p{d}")
                    w_3 = src[:].rearrange(
                        "p (g j) -> p g j", g=NSUB)[:, :, off:off + W]
                    if USE_FUSED_MULT:
                        nc.vector.tensor_mul(
                            p_t[:].rearrange("p (g w) -> p g w", g=NSUB),
                            c1_3, w_3)
                    else:
                        for g in range(NSUB):
                            nc.vector.tensor_mul(
                                p_t[:, g * W:(g + 1) * W],
                                c1_t[:, g * W:(g + 1) * W],
                                src[:, g * WB + off:g * WB + off + W])
                    for g in range(NSUB):
                        lhsT = s_t[:, (g * NOFF + d) * M:(g * NOFF + d + 1) * M]
                        for h in range(NH):
                            nc.tensor.matmul(
                                ps_t[:, h, 0:NCHUNK],
                                lhsT,
                                p_t[:, g * W + h * NCHUNK:
                                    g * W + (h + 1) * NCHUNK],
                                start=(d == 0 and g == 0),
                                stop=(d == NOFF - 1 and g == NSUB - 1),
                            )
                o_t = outs.tile([M, W], f32, tag="o")
                nc.scalar.copy(
                    out=o_t[:].rearrange("p (a b) -> p a b", a=NH),
                    in_=ps_t[:, :, 0:NCHUNK])
                nc.scalar.dma_start(out=out[G], in_=o_t[:])

    nc.compile()
    _BUILT = nc
    return _BUILT


def _prep_c1(c1):
    """[B, H, W, C] f32 -> [B, NG, 128, FW] bf16, partition p = r*32+c."""
    t = c1.reshape(B, NG, NSUB, NR, W, C)           # b G g r w c
    t = t.transpose(0, 1, 3, 5, 2, 4)               # b G r c g w
    return np.ascontiguousarray(t.reshape(B, NG, 128, FW)).astype(bfloat16)


def _prep_warph(warp):
    """[B, H, W, C] f32 -> haloed transposed [B, NG, 128, FWP] bf16."""
    wp = np.zeros((B, H, WB, C), dtype=np.float32)
    wp[:, :, SR:SR + W] = warp
    t = wp.reshape(B, NG, NSUB, NR, WB, C)          # b G g r j c
    t = t.transpose(0, 1, 3, 5, 2, 4)               # b G r c g j
    return np.ascontiguousarray(t.reshape(B, NG, 128, FWP)).astype(bfloat16)


def _make_sones():
    """[128, 20*80] bf16 stationaries; S_{g,d}[(r,c), m] = 1/32 iff
    m == g*20 + r*5 + d."""
    S = np.zeros((128, NSUB * NOFF * M), dtype=np.float32)
    for g in range(NSUB):
        for d in range(NOFF):
            base = (g * NOFF + d) * M
            for r in range(NR):
                S[r * C:(r + 1) * C, base + g * NR * NOFF + r * NOFF + d] = 1.0 / C
    return S.astype(bfloat16)


def _run(c1_full, warph_full, trace=False, **kw):
    from concourse.bass_utils import run_bass_kernel_spmd

    nc = _build()
    c1t = _prep_c1(c1_full)
    sones = _make_sones()
    in_maps = [{"c1t": c1t[i], "wpt": warph_full[i], "sones": sones}
               for i in range(B)]
    return run_bass_kernel_spmd(nc, in_maps, list(range(B)), trace=trace, **kw)


def kernel(c1, warp, search_range):
    assert int(search_range) == SR, f"kernel hardcodes search_range={SR}"
    c1 = np.ascontiguousarray(np.asarray(c1, dtype=np.float32))
    warp = np.ascontiguousarray(np.asarray(warp, dtype=np.float32))
    assert c1.shape == (B, H, W, C) and warp.shape == (B, H, W, C)
    warph = _prep_warph(warp)
    r = _run(c1, warph, trace=False)
    out = np.empty((B, H, W, OUTC), dtype=np.float32)
    out[..., :C] = c1
    for i in range(B):
        cost = np.asarray(r.results[i]["out"], dtype=np.float32)
        # [NG, m=(g,r,d), w] -> [NG, g, r, d, w] -> [NG, g, r, w, d] -> [H,W,5]
        cost = cost.reshape(NG, NSUB, NR, NOFF, W).transpose(0, 1, 2, 4, 3)
        out[i, ..., C:] = cost.reshape(H, W, NOFF)
    return out
